# revision 10
# baseline (speedup 1.0000x reference)
"""Trainium2 Bass kernel for a batched 2D Haar DWT (single level).

Input : x (8, 64, 512, 512) float32
Output: tuple (ll, lh, hl, hh), each (8, 64, 256, 256) float32, matching

    a00 = x[..., 0::2, 0::2]; a01 = x[..., 0::2, 1::2]
    a10 = x[..., 1::2, 0::2]; a11 = x[..., 1::2, 1::2]
    ll = (a00 + a01 + a10 + a11)/2
    lh = (a00 + a01 - a10 - a11)/2
    hl = (a00 - a01 + a10 - a11)/2
    hh = (a00 - a01 - a10 + a11)/2

Sharding: pure data parallel over the batch dim — core i processes x[i]
(64, 512, 512), no communication.

Per-core dataflow (tiles of 2 channel planes, 2 MiB):
  - Partition p holds 4 consecutive input rows per channel (2 row-pairs),
    so the input DMA moves 8 KiB contiguous chunks and each output plane
    leaves 2 KiB contiguous chunks per partition.
  - GpSimd halves the odd rows in place (1-input Q7 op, ~line rate).
  - VectorE scalar_tensor_tensor computes S = 0.5*even + odd' and
    D = 0.5*even - odd' (row butterfly, scale folded in).
  - Column butterfly: ll/hl from S on VectorE, lh/hh from D on GpSimd.
  - Engine roles: SP sequencer only issues loads (runs ahead), ACT
    sequencer only issues stores (runs behind) — no head-of-line beat.
"""

import sys

import numpy as np

for _p in ("/opt/trn_rl_repo",):
    if _p not in sys.path:
        sys.path.insert(0, _p)

from concourse import bacc, mybir  # noqa: E402
from concourse.bass_utils import run_bass_kernel_spmd  # noqa: E402
from concourse.tile import TileContext  # noqa: E402

N_CORES = 8
C, H, W = 64, 512, 512
OUT_KEYS = ("ll", "lh", "hl", "hh")


def build_dwt(c_dim=C, h_dim=H, w_dim=W, bufs=3, cpt=2):
    """Build the per-core Bass module for a (c_dim, h_dim, w_dim) input."""
    f32 = mybir.dt.float32
    r_dim = h_dim // 2          # row pairs per channel
    p_dim = min(r_dim, 128)     # partitions used
    hblk = r_dim // p_dim       # consecutive row-pairs per partition
    assert r_dim % p_dim == 0 and w_dim % 2 == 0 and c_dim % cpt == 0
    wo = w_dim // 2

    nc = bacc.Bacc("TRN2", target_bir_lowering=False, debug=False)
    x = nc.dram_tensor("x", (c_dim, h_dim, w_dim), f32, kind="ExternalInput").ap()
    outs = {
        k: nc.dram_tensor(k, (c_dim, r_dim, wo), f32, kind="ExternalOutput").ap()
        for k in OUT_KEYS
    }
    add = mybir.AluOpType.add
    sub = mybir.AluOpType.subtract
    mult = mybir.AluOpType.mult

    with TileContext(nc) as tc:
        with tc.tile_pool(name="pool", bufs=bufs) as pool:
            for ci in range(c_dim // cpt):
                c0 = ci * cpt
                # --- load cpt channel planes, 4 rows per partition ---
                xt = pool.tile([p_dim, cpt * hblk * 2 * w_dim], f32, tag="xt",
                               name="xt")
                xv = xt.rearrange("p (c h r w) -> p c h r w", c=cpt, h=hblk,
                                  r=2, w=w_dim)
                src = x[c0:c0 + cpt].rearrange("c (p h r) w -> p c h r w",
                                               p=p_dim, h=hblk, r=2)
                nc.sync.dma_start(out=xv, in_=src)
                ev = xv[:, :, :, 0]   # even rows      [p, cpt, hblk, w]
                ov = xv[:, :, :, 1]   # odd rows

                # --- halve odd rows in place (GpSimd, 1-input ~line rate) ---
                nc.gpsimd.tensor_scalar_mul(ov, ov, 0.5)

                # --- row butterfly, 0.5 folded into the even operand ---
                s = pool.tile([p_dim, cpt * hblk * w_dim], f32, tag="s", name="s")
                d = pool.tile([p_dim, cpt * hblk * w_dim], f32, tag="d", name="d")
                sv = s.rearrange("p (c h w) -> p c h w", c=cpt, h=hblk)
                dv = d.rearrange("p (c h w) -> p c h w", c=cpt, h=hblk)
                nc.vector.scalar_tensor_tensor(sv, ev, 0.5, ov, mult, add)
                nc.vector.scalar_tensor_tensor(dv, ev, 0.5, ov, mult, sub)

                # --- column butterfly ---
                se = s.rearrange("p (c h l two) -> p c h l two", c=cpt, h=hblk,
                                 two=2)
                de = d.rearrange("p (c h l two) -> p c h l two", c=cpt, h=hblk,
                                 two=2)
                ob = {
                    k: pool.tile([p_dim, cpt * hblk * wo], f32, tag=f"{k}b",
                                 name=f"{k}b")
                    for k in OUT_KEYS
                }
                obv = {
                    k: t.rearrange("p (c h l) -> p c h l", c=cpt, h=hblk)
                    for k, t in ob.items()
                }
                nc.vector.tensor_add(obv["ll"], se[:, :, :, :, 0],
                                     se[:, :, :, :, 1])
                nc.vector.tensor_sub(obv["hl"], se[:, :, :, :, 0],
                                     se[:, :, :, :, 1])
                nc.gpsimd.tensor_tensor(obv["lh"], de[:, :, :, :, 0],
                                        de[:, :, :, :, 1], add)
                nc.gpsimd.tensor_tensor(obv["hh"], de[:, :, :, :, 0],
                                        de[:, :, :, :, 1], sub)

                # --- store subband planes (ACT sequencer = store-only) ---
                for k in OUT_KEYS:
                    dst = outs[k][c0:c0 + cpt].rearrange(
                        "c (p h) l -> p c h l", p=p_dim, h=hblk)
                    nc.scalar.dma_start(out=dst, in_=obv[k])
    nc.finalize()
    return nc


_CACHE = {}


def _get_nc():
    if "nc" not in _CACHE:
        _CACHE["nc"] = build_dwt()
    return _CACHE["nc"]


def kernel(x):
    x = np.ascontiguousarray(np.asarray(x), dtype=np.float32)
    assert x.shape == (N_CORES, C, H, W), x.shape
    nc = _get_nc()
    in_maps = [{"x": x[i]} for i in range(N_CORES)]
    res = run_bass_kernel_spmd(nc, in_maps, core_ids=list(range(N_CORES)))
    results = res.results
    return tuple(
        np.stack([np.asarray(results[i][k]) for i in range(N_CORES)], axis=0)
        for k in OUT_KEYS
    )


# revision 12
# speedup vs baseline: 3.0471x; 3.0471x over previous
"""Trainium2 Bass kernel for a batched 2D Haar DWT (single level).

Input : x (8, 64, 512, 512) float32
Output: tuple (ll, lh, hl, hh), each (8, 64, 256, 256) float32, matching

    a00 = x[..., 0::2, 0::2]; a01 = x[..., 0::2, 1::2]
    a10 = x[..., 1::2, 0::2]; a11 = x[..., 1::2, 1::2]
    ll = (a00 + a01 + a10 + a11)/2
    lh = (a00 + a01 - a10 - a11)/2
    hl = (a00 - a01 + a10 - a11)/2
    hh = (a00 - a01 - a10 + a11)/2

Sharding: pure data parallel over the batch dim — core i processes x[i]
(64, 512, 512), no communication.

Per-core dataflow (tiles of 2 channel planes, 2 MiB):
  - Partition p holds 4 consecutive input rows per channel (2 row-pairs),
    so the input DMA moves 8 KiB contiguous chunks and each output plane
    leaves 2 KiB contiguous chunks per partition.
  - GpSimd halves the odd rows in place (1-input Q7 op, ~line rate).
  - VectorE scalar_tensor_tensor computes S = 0.5*even + odd' and
    D = 0.5*even - odd' (row butterfly, scale folded in).
  - Column butterfly: ll/hl from S on VectorE, lh/hh from D on GpSimd.
  - Engine roles: SP sequencer only issues loads (runs ahead), ACT
    sequencer only issues stores (runs behind) — no head-of-line beat.
"""

import sys

import numpy as np

for _p in ("/opt/trn_rl_repo",):
    if _p not in sys.path:
        sys.path.insert(0, _p)

from concourse import bacc, mybir  # noqa: E402
from concourse.bass_utils import run_bass_kernel_spmd  # noqa: E402
from concourse.tile import TileContext  # noqa: E402

N_CORES = 8
C, H, W = 64, 512, 512
OUT_KEYS = ("ll", "lh", "hl", "hh")


def build_dwt(c_dim=C, h_dim=H, w_dim=W, bufs=3, cpt=2):
    """Build the per-core Bass module for a (c_dim, h_dim, w_dim) input."""
    f32 = mybir.dt.float32
    r_dim = h_dim // 2          # row pairs per channel
    p_dim = min(r_dim, 128)     # partitions used
    hblk = r_dim // p_dim       # consecutive row-pairs per partition
    assert r_dim % p_dim == 0 and w_dim % 2 == 0 and c_dim % cpt == 0
    wo = w_dim // 2

    nc = bacc.Bacc("TRN2", target_bir_lowering=False, debug=False)
    x = nc.dram_tensor("x", (c_dim, h_dim, w_dim), f32, kind="ExternalInput").ap()
    outs = {
        k: nc.dram_tensor(k, (c_dim, r_dim, wo), f32, kind="ExternalOutput").ap()
        for k in OUT_KEYS
    }
    add = mybir.AluOpType.add
    sub = mybir.AluOpType.subtract
    mult = mybir.AluOpType.mult

    with TileContext(nc) as tc:
        with tc.tile_pool(name="pool", bufs=bufs) as pool:
            for ci in range(c_dim // cpt):
                c0 = ci * cpt
                # --- load cpt channel planes, 4 rows per partition ---
                # q indexes the cpt*hblk row-pair groups; all compute views
                # below keep <=2 free dims (deeper APs hit engine slow paths).
                q_dim = cpt * hblk
                xt = pool.tile([p_dim, q_dim * 2 * w_dim], f32, tag="xt",
                               name="xt")
                xv = xt.rearrange("p (q r w) -> p q r w", q=q_dim, r=2, w=w_dim)
                src = x[c0:c0 + cpt].rearrange("c (p h r) w -> p c h r w",
                                               p=p_dim, h=hblk, r=2)
                nc.sync.dma_start(out=xv.rearrange("p (c h) r w -> p c h r w",
                                                   c=cpt), in_=src)
                ev = xv[:, :, 0]   # even rows  [p, q, w]
                ov = xv[:, :, 1]   # odd rows

                # --- halve odd rows in place (DVE keeps the head role:
                #     giving Pool both head and tail work ping-pongs the
                #     two FIFO sequencers and serializes the pipeline) ---
                nc.vector.tensor_scalar_mul(ov, ov, 0.5)

                # --- row butterfly, 0.5 folded into the even operand ---
                s = pool.tile([p_dim, q_dim * w_dim], f32, tag="s", name="s")
                d = pool.tile([p_dim, q_dim * w_dim], f32, tag="d", name="d")
                sv = s.rearrange("p (q w) -> p q w", q=q_dim)
                dv = d.rearrange("p (q w) -> p q w", q=q_dim)
                nc.vector.scalar_tensor_tensor(sv, ev, 0.5, ov, mult, add)
                nc.vector.scalar_tensor_tensor(dv, ev, 0.5, ov, mult, sub)

                # --- column butterfly ---
                se = s.rearrange("p (q l two) -> p q l two", q=q_dim, two=2)
                de = d.rearrange("p (q l two) -> p q l two", q=q_dim, two=2)
                ob = {
                    k: pool.tile([p_dim, q_dim * wo], f32, tag=f"{k}b",
                                 name=f"{k}b")
                    for k in OUT_KEYS
                }
                obv = {
                    k: t.rearrange("p (q l) -> p q l", q=q_dim)
                    for k, t in ob.items()
                }
                nc.vector.tensor_add(obv["ll"], se[:, :, :, 0], se[:, :, :, 1])
                nc.vector.tensor_sub(obv["hl"], se[:, :, :, 0], se[:, :, :, 1])
                nc.gpsimd.tensor_tensor(obv["lh"], de[:, :, :, 0],
                                        de[:, :, :, 1], add)
                nc.gpsimd.tensor_tensor(obv["hh"], de[:, :, :, 0],
                                        de[:, :, :, 1], sub)

                # --- store subband planes (ACT sequencer = store-only) ---
                for k in OUT_KEYS:
                    dst = outs[k][c0:c0 + cpt].rearrange(
                        "c (p h) l -> p c h l", p=p_dim, h=hblk)
                    src_k = ob[k].rearrange("p (c h l) -> p c h l", c=cpt,
                                            h=hblk)
                    nc.scalar.dma_start(out=dst, in_=src_k)
    nc.finalize()
    return nc


_CACHE = {}


def _get_nc():
    if "nc" not in _CACHE:
        _CACHE["nc"] = build_dwt()
    return _CACHE["nc"]


def kernel(x):
    x = np.ascontiguousarray(np.asarray(x), dtype=np.float32)
    assert x.shape == (N_CORES, C, H, W), x.shape
    nc = _get_nc()
    in_maps = [{"x": x[i]} for i in range(N_CORES)]
    res = run_bass_kernel_spmd(nc, in_maps, core_ids=list(range(N_CORES)))
    results = res.results
    return tuple(
        np.stack([np.asarray(results[i][k]) for i in range(N_CORES)], axis=0)
        for k in OUT_KEYS
    )


# revision 14
# speedup vs baseline: 3.0881x; 1.0134x over previous
"""Trainium2 Bass kernel for a batched 2D Haar DWT (single level).

Input : x (8, 64, 512, 512) float32
Output: tuple (ll, lh, hl, hh), each (8, 64, 256, 256) float32, matching

    a00 = x[..., 0::2, 0::2]; a01 = x[..., 0::2, 1::2]
    a10 = x[..., 1::2, 0::2]; a11 = x[..., 1::2, 1::2]
    ll = (a00 + a01 + a10 + a11)/2
    lh = (a00 + a01 - a10 - a11)/2
    hl = (a00 - a01 + a10 - a11)/2
    hh = (a00 - a01 - a10 + a11)/2

Sharding: pure data parallel over the batch dim — core i processes x[i]
(64, 512, 512), no communication.

Per-core dataflow (tiles of 2 channel planes, 2 MiB):
  - Partition p holds 4 consecutive input rows per channel (2 row-pairs),
    so the input DMA moves 8 KiB contiguous chunks and each output plane
    leaves 2 KiB contiguous chunks per partition.
  - GpSimd halves the odd rows in place (1-input Q7 op, ~line rate).
  - VectorE scalar_tensor_tensor computes S = 0.5*even + odd' and
    D = 0.5*even - odd' (row butterfly, scale folded in).
  - Column butterfly: ll/hl from S on VectorE, lh/hh from D on GpSimd.
  - Engine roles: SP sequencer only issues loads (runs ahead), ACT
    sequencer only issues stores (runs behind) — no head-of-line beat.
"""

import sys

import numpy as np

for _p in ("/opt/trn_rl_repo",):
    if _p not in sys.path:
        sys.path.insert(0, _p)

from concourse import bacc, mybir  # noqa: E402
from concourse.bass_utils import run_bass_kernel_spmd  # noqa: E402
from concourse.tile import TileContext  # noqa: E402

N_CORES = 8
C, H, W = 64, 512, 512
OUT_KEYS = ("ll", "lh", "hl", "hh")


def build_dwt(c_dim=C, h_dim=H, w_dim=W, bufs=3, cpt=2):
    """Build the per-core Bass module for a (c_dim, h_dim, w_dim) input."""
    f32 = mybir.dt.float32
    r_dim = h_dim // 2          # row pairs per channel
    p_dim = min(r_dim, 128)     # partitions used
    hblk = r_dim // p_dim       # consecutive row-pairs per partition
    assert r_dim % p_dim == 0 and w_dim % 2 == 0 and c_dim % cpt == 0
    wo = w_dim // 2

    nc = bacc.Bacc("TRN2", target_bir_lowering=False, debug=False)
    x = nc.dram_tensor("x", (c_dim, h_dim, w_dim), f32, kind="ExternalInput").ap()
    outs = {
        k: nc.dram_tensor(k, (c_dim, r_dim, wo), f32, kind="ExternalOutput").ap()
        for k in OUT_KEYS
    }
    add = mybir.AluOpType.add
    sub = mybir.AluOpType.subtract
    mult = mybir.AluOpType.mult

    with TileContext(nc) as tc:
        with tc.tile_pool(name="pool", bufs=bufs) as pool:
            for ci in range(c_dim // cpt):
                c0 = ci * cpt
                # --- load cpt channel planes, 4 rows per partition ---
                # q indexes the cpt*hblk row-pair groups; all compute views
                # below keep <=2 free dims (deeper APs hit engine slow paths).
                q_dim = cpt * hblk
                xt = pool.tile([p_dim, q_dim * 2 * w_dim], f32, tag="xt",
                               name="xt")
                xv = xt.rearrange("p (q r w) -> p q r w", q=q_dim, r=2, w=w_dim)
                src = x[c0:c0 + cpt].rearrange("c (p h r) w -> p c h r w",
                                               p=p_dim, h=hblk, r=2)
                nc.sync.dma_start(out=xv.rearrange("p (c h) r w -> p c h r w",
                                                   c=cpt), in_=src)
                ev = xv[:, :, 0]   # even rows  [p, q, w]
                ov = xv[:, :, 1]   # odd rows

                # --- halve odd rows into a dense tile (DVE keeps the head
                #     role: giving Pool both head and tail work ping-pongs
                #     the two FIFO sequencers and serializes the pipeline;
                #     dense unit-stride output lets the 2x perf mode engage) ---
                oh = pool.tile([p_dim, q_dim * w_dim], f32, tag="oh", name="oh")
                ohv = oh.rearrange("p (q w) -> p q w", q=q_dim)
                nc.vector.tensor_scalar_mul(ohv, ov, 0.5)

                # --- row butterfly, 0.5 folded into the even operand ---
                s = pool.tile([p_dim, q_dim * w_dim], f32, tag="s", name="s")
                d = pool.tile([p_dim, q_dim * w_dim], f32, tag="d", name="d")
                sv = s.rearrange("p (q w) -> p q w", q=q_dim)
                dv = d.rearrange("p (q w) -> p q w", q=q_dim)
                nc.vector.scalar_tensor_tensor(sv, ev, 0.5, ohv, mult, add)
                nc.vector.scalar_tensor_tensor(dv, ev, 0.5, ohv, mult, sub)

                # --- column butterfly ---
                se = s.rearrange("p (q l two) -> p q l two", q=q_dim, two=2)
                de = d.rearrange("p (q l two) -> p q l two", q=q_dim, two=2)
                ob = {
                    k: pool.tile([p_dim, q_dim * wo], f32, tag=f"{k}b",
                                 name=f"{k}b")
                    for k in OUT_KEYS
                }
                obv = {
                    k: t.rearrange("p (q l) -> p q l", q=q_dim)
                    for k, t in ob.items()
                }
                nc.vector.tensor_add(obv["ll"], se[:, :, :, 0], se[:, :, :, 1])
                nc.gpsimd.tensor_tensor(obv["hl"], se[:, :, :, 0],
                                        se[:, :, :, 1], sub)
                nc.gpsimd.tensor_tensor(obv["lh"], de[:, :, :, 0],
                                        de[:, :, :, 1], add)
                nc.gpsimd.tensor_tensor(obv["hh"], de[:, :, :, 0],
                                        de[:, :, :, 1], sub)

                # --- store subband planes (ACT sequencer = store-only) ---
                for k in OUT_KEYS:
                    dst = outs[k][c0:c0 + cpt].rearrange(
                        "c (p h) l -> p c h l", p=p_dim, h=hblk)
                    src_k = ob[k].rearrange("p (c h l) -> p c h l", c=cpt,
                                            h=hblk)
                    nc.scalar.dma_start(out=dst, in_=src_k)
    nc.finalize()
    return nc


_CACHE = {}


def _get_nc():
    if "nc" not in _CACHE:
        _CACHE["nc"] = build_dwt()
    return _CACHE["nc"]


def kernel(x):
    x = np.ascontiguousarray(np.asarray(x), dtype=np.float32)
    assert x.shape == (N_CORES, C, H, W), x.shape
    nc = _get_nc()
    in_maps = [{"x": x[i]} for i in range(N_CORES)]
    res = run_bass_kernel_spmd(nc, in_maps, core_ids=list(range(N_CORES)))
    results = res.results
    return tuple(
        np.stack([np.asarray(results[i][k]) for i in range(N_CORES)], axis=0)
        for k in OUT_KEYS
    )


# revision 16
# speedup vs baseline: 3.1204x; 1.0105x over previous
"""Trainium2 Bass kernel for a batched 2D Haar DWT (single level).

Input : x (8, 64, 512, 512) float32
Output: tuple (ll, lh, hl, hh), each (8, 64, 256, 256) float32, matching

    a00 = x[..., 0::2, 0::2]; a01 = x[..., 0::2, 1::2]
    a10 = x[..., 1::2, 0::2]; a11 = x[..., 1::2, 1::2]
    ll = (a00 + a01 + a10 + a11)/2
    lh = (a00 + a01 - a10 - a11)/2
    hl = (a00 - a01 + a10 - a11)/2
    hh = (a00 - a01 - a10 + a11)/2

Sharding: pure data parallel over the batch dim — core i processes x[i]
(64, 512, 512), no communication.

Per-core dataflow (tiles of 2 channel planes, 2 MiB):
  - Partition p holds 4 consecutive input rows per channel (2 row-pairs),
    so the input DMA moves 8 KiB contiguous chunks and each output plane
    leaves 2 KiB contiguous chunks per partition.
  - GpSimd halves the odd rows in place (1-input Q7 op, ~line rate).
  - VectorE scalar_tensor_tensor computes S = 0.5*even + odd' and
    D = 0.5*even - odd' (row butterfly, scale folded in).
  - Column butterfly: ll/hl from S on VectorE, lh/hh from D on GpSimd.
  - Engine roles: SP sequencer only issues loads (runs ahead), ACT
    sequencer only issues stores (runs behind) — no head-of-line beat.
"""

import sys

import numpy as np

for _p in ("/opt/trn_rl_repo",):
    if _p not in sys.path:
        sys.path.insert(0, _p)

from concourse import bacc, mybir  # noqa: E402
from concourse.bass_utils import run_bass_kernel_spmd  # noqa: E402
from concourse.tile import TileContext  # noqa: E402

N_CORES = 8
C, H, W = 64, 512, 512
OUT_KEYS = ("ll", "lh", "hl", "hh")


def build_dwt(c_dim=C, h_dim=H, w_dim=W, bufs=3, cpt=2):
    """Build the per-core Bass module for a (c_dim, h_dim, w_dim) input."""
    f32 = mybir.dt.float32
    r_dim = h_dim // 2          # row pairs per channel
    p_dim = min(r_dim, 128)     # partitions used
    hblk = r_dim // p_dim       # consecutive row-pairs per partition
    assert r_dim % p_dim == 0 and w_dim % 2 == 0 and c_dim % cpt == 0
    wo = w_dim // 2

    nc = bacc.Bacc("TRN2", target_bir_lowering=False, debug=False)
    x = nc.dram_tensor("x", (c_dim, h_dim, w_dim), f32, kind="ExternalInput").ap()
    outs = {
        k: nc.dram_tensor(k, (c_dim, r_dim, wo), f32, kind="ExternalOutput").ap()
        for k in OUT_KEYS
    }
    add = mybir.AluOpType.add
    sub = mybir.AluOpType.subtract
    mult = mybir.AluOpType.mult

    with TileContext(nc) as tc:
        with tc.tile_pool(name="pool", bufs=bufs) as pool:
            for ci in range(c_dim // cpt):
                c0 = ci * cpt
                # --- load cpt channel planes, 4 rows per partition ---
                # q indexes the cpt*hblk row-pair groups; all compute views
                # below keep <=2 free dims (deeper APs hit engine slow paths).
                q_dim = cpt * hblk
                xt = pool.tile([p_dim, q_dim * 2 * w_dim], f32, tag="xt",
                               name="xt", bufs=4)
                xv = xt.rearrange("p (q r w) -> p q r w", q=q_dim, r=2, w=w_dim)
                src = x[c0:c0 + cpt].rearrange("c (p h r) w -> p c h r w",
                                               p=p_dim, h=hblk, r=2)
                nc.sync.dma_start(out=xv.rearrange("p (c h) r w -> p c h r w",
                                                   c=cpt), in_=src)
                ev = xv[:, :, 0]   # even rows  [p, q, w]
                ov = xv[:, :, 1]   # odd rows

                # --- halve odd rows in place (DVE keeps the head role:
                #     giving Pool both head and tail work ping-pongs the
                #     two FIFO sequencers and serializes the pipeline) ---
                nc.vector.tensor_scalar_mul(ov, ov, 0.5)

                # --- row butterfly, 0.5 folded into the even operand ---
                s = pool.tile([p_dim, q_dim * w_dim], f32, tag="s", name="s")
                d = pool.tile([p_dim, q_dim * w_dim], f32, tag="d", name="d")
                sv = s.rearrange("p (q w) -> p q w", q=q_dim)
                dv = d.rearrange("p (q w) -> p q w", q=q_dim)
                nc.vector.scalar_tensor_tensor(sv, ev, 0.5, ov, mult, add)
                nc.vector.scalar_tensor_tensor(dv, ev, 0.5, ov, mult, sub)

                # --- column butterfly ---
                se = s.rearrange("p (q l two) -> p q l two", q=q_dim, two=2)
                de = d.rearrange("p (q l two) -> p q l two", q=q_dim, two=2)
                ob = {
                    k: pool.tile([p_dim, q_dim * wo], f32, tag=f"{k}b",
                                 name=f"{k}b")
                    for k in OUT_KEYS
                }
                obv = {
                    k: t.rearrange("p (q l) -> p q l", q=q_dim)
                    for k, t in ob.items()
                }
                nc.vector.tensor_add(obv["ll"], se[:, :, :, 0], se[:, :, :, 1])
                nc.gpsimd.tensor_tensor(obv["hl"], se[:, :, :, 0],
                                        se[:, :, :, 1], sub)
                nc.gpsimd.tensor_tensor(obv["lh"], de[:, :, :, 0],
                                        de[:, :, :, 1], add)
                nc.gpsimd.tensor_tensor(obv["hh"], de[:, :, :, 0],
                                        de[:, :, :, 1], sub)

                # --- store subband planes (ACT sequencer = store-only) ---
                for k in OUT_KEYS:
                    dst = outs[k][c0:c0 + cpt].rearrange(
                        "c (p h) l -> p c h l", p=p_dim, h=hblk)
                    src_k = ob[k].rearrange("p (c h l) -> p c h l", c=cpt,
                                            h=hblk)
                    nc.scalar.dma_start(out=dst, in_=src_k)
    nc.finalize()
    return nc


_CACHE = {}


def _get_nc():
    if "nc" not in _CACHE:
        _CACHE["nc"] = build_dwt()
    return _CACHE["nc"]


def kernel(x):
    x = np.ascontiguousarray(np.asarray(x), dtype=np.float32)
    assert x.shape == (N_CORES, C, H, W), x.shape
    nc = _get_nc()
    in_maps = [{"x": x[i]} for i in range(N_CORES)]
    res = run_bass_kernel_spmd(nc, in_maps, core_ids=list(range(N_CORES)))
    results = res.results
    return tuple(
        np.stack([np.asarray(results[i][k]) for i in range(N_CORES)], axis=0)
        for k in OUT_KEYS
    )
